# revision 7
# baseline (speedup 1.0000x reference)
"""IntervalLoss kernel for Trainium2, 8 NeuronCores, data-parallel over batch.

Math (per element, exact f32 semantics of the reference):
  loss = (p-t)^2 by default; if |t-c_j| < 0.01 for one of 11 interval specs
  (c, lo, hi), loss = relu(lo-p)^2 + relu(p-hi)^2.

Kernel works in u-space (x40): U=40t, P=40p. All band boundaries/values are
then small integers. r = round_to_nearest(U) via the DVE f32->int32 cast
(verified RNE on HW). In-band <=> |U-r| < 0.4 and r in K (11 integer
centers). lo/hi values accumulate via is_equal masks. Final per-element:
  a = (U-P) + VL - S*U      (= LO40 - P40 in band, (U-P) off band)
  b-side t1 = (U-P) + VH - S*U, loss_b = relu(-t1)^2
  sum relu(a)^2 + relu(-t1)^2, divided by 1600*N on the host.
"""

import os
import sys

import numpy as np

for _p in ("/opt/trn_rl_repo", "/root/.axon_site/_ro/trn_rl_repo"):
    if _p not in sys.path and os.path.isdir(_p):
        sys.path.append(_p)

from concourse import bass, mybir  # noqa: E402
from concourse.bass_utils import run_bass_kernel_spmd  # noqa: E402

N_CORES = 8
B, C, H, W = 32, 1, 1024, 1024
PER_CORE = B // N_CORES  # 4 batches per core
P_DIM = 128
F_TOTAL = PER_CORE * C * H * W // P_DIM  # 32768
F_TILE = 2048
N_TILES = F_TOTAL // F_TILE  # 16

# (center, lo, hi) * 40 -> integers
RANGES = [
    (0.05, 0.0, 0.1), (0.125, 0.0, 0.15), (0.225, 0.15, 0.3),
    (0.4, 0.3, 0.7), (0.5, 0.3, 0.7), (0.6, 0.3, 0.7),
    (0.75, 0.7, 1.2), (0.95, 0.7, 1.2),
    (1.6, 1.2, 2.5), (2.0, 1.2, 3.0), (2.5, 1.2, 5.0),
]
K40 = [round(c * 40) for c, _, _ in RANGES]     # [2,5,9,16,20,24,30,38,64,80,100]
LO40 = [round(lo * 40) for _, lo, _ in RANGES]  # [0,0,6,12,12,12,28,28,48,48,48]
HI40 = [round(hi * 40) for _, _, hi in RANGES]  # [4,6,12,28,28,28,48,48,100,120,200]

_F32 = mybir.dt.float32
_I32 = mybir.dt.int32
_OP = mybir.AluOpType


def _build_nc():
    nc = bass.Bass()
    pred_ext = nc.declare_dram_parameter("pred", [P_DIM, F_TOTAL], _F32, isOutput=False)
    targ_ext = nc.declare_dram_parameter("target", [P_DIM, F_TOTAL], _F32, isOutput=False)
    out_ext = nc.declare_dram_parameter("out", [P_DIM, 2 * N_TILES], _F32, isOutput=True)

    sb = lambda name, shape, dt=_F32: nc.alloc_sbuf_tensor(name, shape, dt).ap()
    pt = [sb(f"pt{i}", [P_DIM, F_TILE]) for i in range(2)]
    tt = [sb(f"tt{i}", [P_DIM, F_TILE]) for i in range(2)]
    Ut = [sb(f"Ut{i}", [P_DIM, F_TILE]) for i in range(2)]
    RI = sb("RI", [P_DIM, F_TILE], _I32)
    RF = sb("RF", [P_DIM, F_TILE])
    WB = sb("WB", [P_DIM, F_TILE])
    WC = sb("WC", [P_DIM, F_TILE])
    WL = [sb(f"WL{i}", [P_DIM, F_TILE]) for i in range(2)]
    WH = [sb(f"WH{i}", [P_DIM, F_TILE]) for i in range(2)]
    GA = [sb(f"GA{i}", [P_DIM, F_TILE]) for i in range(2)]
    GT = [sb(f"GT{i}", [P_DIM, F_TILE]) for i in range(2)]
    WD = sb("WD", [P_DIM, F_TILE])
    _BF16 = mybir.dt.bfloat16
    RQb = sb("RQb", [P_DIM, F_TILE], _BF16)
    WEb = sb("WEb", [P_DIM, F_TILE], _BF16)
    WLb = sb("WLb", [P_DIM, F_TILE], _BF16)
    WHb = sb("WHb", [P_DIM, F_TILE], _BF16)
    acc = sb("acc", [P_DIM, 2 * N_TILES])

    with nc.Block() as block, \
            nc.semaphore("dma_sem") as dma_sem, \
            nc.semaphore("act_done") as act_done, \
            nc.semaphore("bands_done") as bands_done, \
            nc.semaphore("gp_done") as gp_done, \
            nc.semaphore("tail_done") as tail_done:

        @block.sync
        def _(sync):
            for i in range(N_TILES):
                if i >= 2:
                    # input buffers freed once GPSIMD has consumed tile i-2
                    # (pt is read by the GPSIMD tail; tt by ACT before that)
                    sync.wait_ge(gp_done, i - 1)
                b = i % 2
                sl = slice(i * F_TILE, (i + 1) * F_TILE)
                sync.dma_start(out=pt[b][:], in_=pred_ext[:, sl]).then_inc(dma_sem, 16)
                sync.dma_start(out=tt[b][:], in_=targ_ext[:, sl]).then_inc(dma_sem, 16)

        @block.scalar
        def _(act):
            for i in range(N_TILES):
                act.wait_ge(dma_sem, 32 * (i + 1))
                if i >= 2:
                    # U buffer freed once GPSIMD finished tile i-2
                    act.wait_ge(gp_done, i - 1)
                b = i % 2
                act.mul(Ut[b][:], tt[b][:], 40.0)
                act.drain()
                act.sem_inc(act_done, 1)

        @block.vector
        def _(v):
            def tail(i):
                # Sum relu(a)^2 and relu(-t1)^2 for tile i (a=GA, t1=GT from GPSIMD)
                v.wait_ge(gp_done, i + 1)
                bb = i % 2
                v.scalar_tensor_tensor(out=WB[:], in0=GA[bb][:], scalar=0.0,
                                       in1=GA[bb][:], op0=_OP.max, op1=_OP.mult,
                                       accum_out=acc[:, 2 * i:2 * i + 1])
                v.scalar_tensor_tensor(out=WC[:], in0=GT[bb][:], scalar=0.0,
                                       in1=GT[bb][:], op0=_OP.min, op1=_OP.mult,
                                       accum_out=acc[:, 2 * i + 1:2 * i + 2])
                v.drain()
                v.sem_inc(tail_done, 1)

            for i in range(N_TILES):
                v.wait_ge(act_done, i + 1)
                if i >= 2:
                    # WL/WH buffers freed once GPSIMD finished tile i-2
                    v.wait_ge(gp_done, i - 1)
                b = i % 2
                U = Ut[b]
                # r = rne(U)  (f32->i32 cast is round-to-nearest-even)
                v.tensor_copy(RI[:], U[:])
                v.tensor_copy(RF[:], RI[:])
                # |U - r| < 0.4 ?
                v.tensor_sub(WB[:], U[:], RF[:])
                v.tensor_scalar(out=WB.bitcast(_I32)[:], in0=WB.bitcast(_I32)[:],
                                scalar1=0x7FFFFFFF, scalar2=None, op0=_OP.bitwise_and)
                v.tensor_scalar(out=WB[:], in0=WB[:], scalar1=0.4, scalar2=None,
                                op0=_OP.is_lt)
                # push off-band r out of range: RQ = RF + (INB-1)*1e6
                v.tensor_scalar(out=WC[:], in0=WB[:], scalar1=1.0, scalar2=1e6,
                                op0=_OP.subtract, op1=_OP.mult)
                v.tensor_add(RF[:], RF[:], WC[:])
                # band block in bf16: all values are small integers -> exact.
                # bf16 tensor_scalar runs 4x, bf16 STT 2x on the DVE.
                v.tensor_copy(RQb[:], RF[:])
                first_lo = True
                first_hi = True
                for j in range(len(K40)):
                    kf = float(K40[j])
                    if first_hi:
                        v.tensor_scalar(out=WHb[:], in0=RQb[:], scalar1=kf,
                                        scalar2=float(HI40[j]), op0=_OP.is_equal,
                                        op1=_OP.mult)
                        first_hi = False
                        continue  # band 0 has lo=0, no VL term needed
                    v.tensor_scalar(out=WEb[:], in0=RQb[:], scalar1=kf, scalar2=None,
                                    op0=_OP.is_equal)
                    v.scalar_tensor_tensor(out=WHb[:], in0=WEb[:], scalar=float(HI40[j]),
                                           in1=WHb[:], op0=_OP.mult, op1=_OP.add)
                    if LO40[j] > 0:
                        if first_lo:
                            v.tensor_scalar(out=WLb[:], in0=WEb[:], scalar1=float(LO40[j]),
                                            scalar2=None, op0=_OP.mult)
                            first_lo = False
                        else:
                            v.scalar_tensor_tensor(out=WLb[:], in0=WEb[:],
                                                   scalar=float(LO40[j]), in1=WLb[:],
                                                   op0=_OP.mult, op1=_OP.add)
                v.tensor_copy(WL[b][:], WLb[:])
                v.tensor_copy(WH[b][:], WHb[:])
                v.drain()
                v.sem_inc(bands_done, 1)
                if i >= 1:
                    tail(i - 1)
            tail(N_TILES - 1)

        @block.gpsimd
        def _(g):
            for i in range(N_TILES):
                g.wait_ge(bands_done, i + 1)
                if i >= 2:
                    # GA/GT buffers freed once DVE tail consumed tile i-2
                    g.wait_ge(tail_done, i - 1)
                b = i % 2
                U = Ut[b]
                # S = (VH>0); SU = S*U; GL = VL-SU; GH = VH-SU (in place)
                g.tensor_scalar(out=WD[:], in0=WH[b][:], scalar1=0.0, scalar2=None,
                                op0=_OP.is_gt)
                g.tensor_mul(WD[:], WD[:], U[:])
                g.tensor_sub(WL[b][:], WL[b][:], WD[:])
                g.tensor_sub(WH[b][:], WH[b][:], WD[:])
                # dneg = 40*p - U = -(U-P40); a = GL - dneg; t1 = GH - dneg
                g.tensor_scalar_mul(WD[:], pt[b][:], 40.0)
                g.tensor_sub(WD[:], WD[:], U[:])
                g.tensor_sub(GA[b][:], WL[b][:], WD[:])
                g.tensor_sub(GT[b][:], WH[b][:], WD[:])
                g.drain()
                g.sem_inc(gp_done, 1)
            g.wait_ge(tail_done, N_TILES)
            g.dma_start(out=out_ext[:], in_=acc[:]).then_inc(dma_sem, 16)
            g.wait_ge(dma_sem, 32 * N_TILES + 16)

    return nc


_NC_CACHE = None


def kernel(pred: np.ndarray, target: np.ndarray) -> np.ndarray:
    global _NC_CACHE
    if _NC_CACHE is None:
        _NC_CACHE = _build_nc()
    nc = _NC_CACHE

    pred = np.ascontiguousarray(pred, dtype=np.float32)
    target = np.ascontiguousarray(target, dtype=np.float32)

    in_maps = []
    for i in range(N_CORES):
        ps = pred[i * PER_CORE:(i + 1) * PER_CORE].reshape(P_DIM, F_TOTAL)
        ts = target[i * PER_CORE:(i + 1) * PER_CORE].reshape(P_DIM, F_TOTAL)
        in_maps.append({"pred": ps, "target": ts})

    res = run_bass_kernel_spmd(nc, in_maps, list(range(N_CORES)))

    total = np.float64(0.0)
    for i in range(N_CORES):
        total += res.results[i]["out"].astype(np.float64).sum()
    n_elems = float(B * C * H * W)
    mean = total / (n_elems * 1600.0)  # 1600 = 40^2 u-space scaling
    return np.float32(mean)
